# revision 1
# baseline (speedup 1.0000x reference)
"""CrossViewAttention Trainium2 kernel.

Strategy: shard the Q=2500 query positions across 8 cores (Q padded to
2560 = 8*320). Softmax is over NK, which stays local per core, so no
collectives are needed. Per core everything runs in a "transposed"
layout: logits^T [NK_tile=128 partitions, Q=320 free] so that QK^T,
the softmax normalizer (one-hot ones-matmul), and attn@V all run on the
PE without any attention-matrix transposes. Masking uses
e = exp(logits*W*vis)*vis, which matches the reference's finfo.min
trick to float precision because logits are tiny. The softmax
normalizer is folded in after attn@V, and the per-head output
projection is accumulated directly from per-head tiles so no
partition-shifted engine ops are needed anywhere.
"""

import sys

if "/opt/trn_rl_repo" not in sys.path:
    sys.path.insert(0, "/opt/trn_rl_repo")

import numpy as np
import ml_dtypes

import concourse.bass as bass
import concourse.bacc as bacc_mod
import concourse.mybir as mybir
from concourse.tile import TileContext
from concourse.masks import make_identity

# problem constants (hardcoded per harness contract)
HEADS = 4
DH = 32
D = 128
EPS = 1e-5
HB = WB = 50
Q = HB * WB            # 2500
NVIEW, KH, KW = 6, 24, 44
NK = NVIEW * KH * KW   # 6336
NCORES = 8
QC = 320               # queries per core (Q padded to 2560)
QPAD = NCORES * QC
NKP = 6400             # NK padded to 50*128
NKT = NKP // 128       # 50 nk tiles
SCALE = DH ** -0.5

F32 = mybir.dt.float32
BF16 = mybir.dt.bfloat16
X = mybir.AxisListType.X
AF = mybir.ActivationFunctionType

_CACHE = {}


def _ln_partition_stats(nc, pool, pool1, ps_pool, ps_tag, pbc_pool, pbc_tag,
                        ones_col, ones_row, x_sbuf, out, g_ap, b_ap):
    """LayerNorm of x [128 partitions, Qf free] over the PARTITION dim.

    Column stats via ones-matmuls, broadcast back via K=1 matmuls, then
    out = ((x - mean) * rstd) * g + b with per-partition g/b on ACT.
    """
    Qf = x_sbuf.shape[-1]
    ps1 = ps_pool.tile([1, Qf], F32, tag=ps_tag)
    nc.tensor.matmul(ps1, ones_col, x_sbuf, start=True, stop=True)
    sq = pool1.tile([128, Qf], F32, tag="lnsq")
    ps2 = ps_pool.tile([1, Qf], F32, tag=ps_tag)
    nc.scalar.activation(sq, x_sbuf, AF.Square)
    nc.tensor.matmul(ps2, ones_col, sq, start=True, stop=True)
    mean = pool.tile([1, Qf], F32, tag="lnmean")
    ex2 = pool.tile([1, Qf], F32, tag="lnex2")
    nc.scalar.mul(mean, ps1, 1.0 / 128.0)
    nc.scalar.mul(ex2, ps2, 1.0 / 128.0)
    m2 = pool.tile([1, Qf], F32, tag="lnm2")
    nc.vector.tensor_mul(out=m2, in0=mean, in1=mean)
    var = pool.tile([1, Qf], F32, tag="lnvar")
    nc.vector.tensor_tensor(out=var, in0=ex2, in1=m2, op=mybir.AluOpType.subtract)
    std = pool.tile([1, Qf], F32, tag="lnstd")
    nc.scalar.activation(std, var, AF.Sqrt, bias=EPS)
    rstd = pool.tile([1, Qf], F32, tag="lnrstd")
    nc.vector.reciprocal(rstd, std)
    nmr = pool.tile([1, Qf], F32, tag="lnnmr")
    nc.vector.tensor_mul(out=nmr, in0=mean, in1=rstd)
    nc.scalar.mul(nmr, nmr, -1.0)
    pA = pbc_pool.tile([128, Qf], F32, tag=pbc_tag)
    pC = pbc_pool.tile([128, Qf], F32, tag=pbc_tag)
    nc.tensor.matmul(pA, ones_row, rstd, start=True, stop=True)
    nc.tensor.matmul(pC, ones_row, nmr, start=True, stop=True)
    t1 = pool1.tile([128, Qf], F32, tag="lnt1")
    nc.vector.tensor_mul(out=t1, in0=x_sbuf, in1=pA)
    t2 = pool1.tile([128, Qf], F32, tag="lnt2")
    nc.vector.tensor_add(out=t2, in0=t1, in1=pC)
    nc.scalar.activation(out, t2, AF.Identity, scale=g_ap, bias=b_ap)


def _build():
    if "nc" in _CACHE:
        return _CACHE["nc"]
    nc = bacc_mod.Bacc()

    # ---- I/O ----
    qT = nc.dram_tensor("qT", [D, QC], F32, kind="ExternalInput")
    kR = nc.dram_tensor("kR", [NKP, D], F32, kind="ExternalInput")
    vR = nc.dram_tensor("vR", [NKP, D], F32, kind="ExternalInput")
    Wt = nc.dram_tensor("Wt", [NKT, 128, QC], BF16, kind="ExternalInput")
    Cm = nc.dram_tensor("Cm", [NKT, 128, QC], BF16, kind="ExternalInput")
    skipT = nc.dram_tensor("skipT", [D, QC], F32, kind="ExternalInput")
    wqT = nc.dram_tensor("wqT", [D, D], BF16, kind="ExternalInput")
    wkT = nc.dram_tensor("wkT", [D, D], BF16, kind="ExternalInput")
    wvT = nc.dram_tensor("wvT", [D, D], BF16, kind="ExternalInput")
    bqm = nc.dram_tensor("bqm", [64, 2], F32, kind="ExternalInput")
    bkm = nc.dram_tensor("bkm", [64, 2], F32, kind="ExternalInput")
    wprojTm = nc.dram_tensor("wprojTm", [DH, HEADS, D], BF16, kind="ExternalInput")
    bprojv = nc.dram_tensor("bprojv", [D, 1], F32, kind="ExternalInput")
    pre_gv = nc.dram_tensor("pre_gv", [D, 1], F32, kind="ExternalInput")
    pre_bv = nc.dram_tensor("pre_bv", [D, 1], F32, kind="ExternalInput")
    w1T = nc.dram_tensor("w1T", [D, 2 * D], BF16, kind="ExternalInput")
    b1m = nc.dram_tensor("b1m", [D, 2], F32, kind="ExternalInput")
    w2Td = nc.dram_tensor("w2Td", [2, D, D], BF16, kind="ExternalInput")
    b2v = nc.dram_tensor("b2v", [D, 1], F32, kind="ExternalInput")
    post_gv = nc.dram_tensor("post_gv", [D, 1], F32, kind="ExternalInput")
    post_bv = nc.dram_tensor("post_bv", [D, 1], F32, kind="ExternalInput")
    outT = nc.dram_tensor("outT", [D, QC], F32, kind="ExternalOutput")

    with TileContext(nc) as tc:
        with tc.tile_pool(name="const", bufs=1) as cpool, \
             tc.tile_pool(name="big", bufs=1) as bigpool, \
             tc.tile_pool(name="work", bufs=3) as work, \
             tc.tile_pool(name="io", bufs=1) as io:

            # ---- constants ----
            ident = cpool.tile([128, 128], BF16)
            make_identity(nc, ident)
            ones_col = cpool.tile([128, 1], F32)
            nc.any.memset(ones_col, 1.0)
            ones_row = cpool.tile([1, 128], F32)
            nc.any.memset(ones_row, 1.0)
            ones_rbf = cpool.tile([1, 128], BF16)
            nc.any.memset(ones_rbf, 1.0)
            zero_c = cpool.tile([128, 1], F32)
            nc.any.memset(zero_c, 0.0)
            nc.const_aps.aps[(F32, 0.0)] = zero_c[:]
            eps_c = cpool.tile([128, 1], F32)
            nc.any.memset(eps_c, EPS)
            nc.const_aps.aps[(F32, EPS)] = eps_c[:]
            ones6432 = cpool.tile([64, 32], F32)
            nc.any.memset(ones6432, 1.0)

            def load_const(dram, shape, dt):
                t = cpool.tile(shape, dt, tag="c_" + dram.name)
                nc.sync.dma_start(t, dram[...])
                return t

            wq_s = load_const(wqT, [D, D], BF16)
            wk_s = load_const(wkT, [D, D], BF16)
            wv_s = load_const(wvT, [D, D], BF16)
            bq_s = load_const(bqm, [64, 2], F32)
            bk_s = load_const(bkm, [64, 2], F32)
            wproj_s = load_const(wprojTm, [DH, HEADS, D], BF16)
            bproj_s = load_const(bprojv, [D, 1], F32)
            preg_s = load_const(pre_gv, [D, 1], F32)
            preb_s = load_const(pre_bv, [D, 1], F32)
            w1_s = load_const(w1T, [D, 2 * D], BF16)
            b1_s = load_const(b1m, [D, 2], F32)
            w2_s = cpool.tile([D, 2, D], BF16)
            nc.sync.dma_start(w2_s[:, 0, :], w2Td[0])
            nc.sync.dma_start(w2_s[:, 1, :], w2Td[1])
            b2_s = load_const(b2v, [D, 1], F32)
            postg_s = load_const(post_gv, [D, 1], F32)
            postb_s = load_const(post_bv, [D, 1], F32)

            # ---- resident tensors ----
            # kfT/qfT split into lo/hi 64-partition halves so every
            # per-head [32, ...] slice has base partition 0 or 32.
            kf_lo = bigpool.tile([64, NKT, 128], BF16)
            kf_hi = bigpool.tile([64, NKT, 128], BF16)
            qf_lo = bigpool.tile([64, QC], BF16)
            qf_hi = bigpool.tile([64, QC], BF16)
            vf = bigpool.tile([128, NKT, HEADS, DH + 1], BF16)  # [nk,nkt,h,dh+1]
            nc.any.memset(vf[:, :, :, DH], 1.0)
            Wsb = bigpool.tile([128, NKT, QC], BF16)
            Csb = bigpool.tile([128, NKT, QC], BF16)

            # ---- k/v prep (row LayerNorm + projection), chunked ----
            CH = 10
            with tc.tile_pool(name="psum_prep", bufs=2, space="PSUM") as ppre, \
                 tc.tile_pool(name="prep2", bufs=2) as prep2:
                for which in ("k", "v"):
                    src_d = kR if which == "k" else vR
                    for c0 in range(0, NKT, CH):
                        raw = prep2.tile([128, CH, D], F32, tag="kvraw")
                        nc.sync.dma_start(
                            raw, src_d[c0 * 128:(c0 + CH) * 128, :].rearrange(
                                "(t p) d -> p t d", p=128))
                        s1 = work.tile([128, CH], F32, tag="s1")
                        nc.vector.reduce_sum(s1, raw, axis=X)
                        sq = prep2.tile([128, CH, D], F32, tag="big_scratch")
                        nc.vector.tensor_mul(out=sq, in0=raw, in1=raw)
                        s2 = work.tile([128, CH], F32, tag="s2")
                        nc.vector.reduce_sum(s2, sq, axis=X)
                        meanN = work.tile([128, CH], F32, tag="meanN")
                        nc.scalar.mul(meanN, s1, -1.0 / D)
                        ex2 = work.tile([128, CH], F32, tag="ex2")
                        nc.scalar.mul(ex2, s2, 1.0 / D)
                        m2 = work.tile([128, CH], F32, tag="m2")
                        nc.vector.tensor_mul(out=m2, in0=meanN, in1=meanN)
                        var = work.tile([128, CH], F32, tag="var")
                        nc.vector.tensor_tensor(out=var, in0=ex2, in1=m2,
                                                op=mybir.AluOpType.subtract)
                        std = work.tile([128, CH], F32, tag="std")
                        nc.scalar.activation(std, var, AF.Sqrt, bias=EPS)
                        rstd = work.tile([128, CH], F32, tag="rstd")
                        nc.vector.reciprocal(rstd, std)
                        nmr = work.tile([128, CH], F32, tag="nmr")
                        nc.vector.tensor_mul(out=nmr, in0=meanN, in1=rstd)
                        t1 = prep2.tile([128, CH, D], F32, tag="big_scratch")
                        nc.vector.tensor_mul(
                            out=t1, in0=raw,
                            in1=rstd[:, :, None].to_broadcast((128, CH, D)))
                        kn = prep2.tile([128, CH, D], BF16, tag="knc")
                        nc.vector.tensor_add(
                            out=kn, in0=t1,
                            in1=nmr[:, :, None].to_broadcast((128, CH, D)))
                        for i in range(CH):
                            t = c0 + i
                            pt = ppre.tile([128, 128], BF16, tag="pt")
                            nc.tensor.transpose(pt, kn[:, i, :], ident)
                            normT = work.tile([128, D], BF16, tag="normT")
                            nc.any.tensor_copy(out=normT, in_=pt)
                            if which == "k":
                                pk_lo = ppre.tile([64, 128], F32, tag="pkv")
                                nc.tensor.matmul(pk_lo, wk_s[:, 0:64], normT,
                                                 start=True, stop=True)
                                nc.scalar.activation(kf_lo[:, t, :], pk_lo,
                                                     AF.Identity,
                                                     bias=bk_s[:, 0:1])
                                pk_hi = ppre.tile([64, 128], F32, tag="pkv")
                                nc.tensor.matmul(pk_hi, wk_s[:, 64:128], normT,
                                                 start=True, stop=True)
                                nc.scalar.activation(kf_hi[:, t, :], pk_hi,
                                                     AF.Identity,
                                                     bias=bk_s[:, 1:2])
                            else:
                                pv = ppre.tile([128, 128], F32, tag="pv")
                                nc.tensor.matmul(pv, normT, wv_s, start=True,
                                                 stop=True)
                                nc.any.tensor_copy(
                                    out=vf[:, t, :, :DH],
                                    in_=pv.rearrange("p (h e) -> p h e",
                                                     h=HEADS))

                # load the big mask tensors after prep DMAs are queued
                for t in range(NKT):
                    nc.sync.dma_start(Wsb[:, t, :], Wt[t])
                    nc.sync.dma_start(Csb[:, t, :], Cm[t])

            # ---- q prep ----
            with tc.tile_pool(name="psum_q", bufs=2, space="PSUM") as pqp:
                qsb = io.tile([D, QC], F32, tag="qsb")
                nc.sync.dma_start(qsb, qT[...])
                qn01 = work.tile([D, QC], BF16, tag="qn01")
                _ln_partition_stats(nc, work, io, pqp, "ps", pqp, "pbc",
                                    ones_col, ones_row, qsb, qn01, 1.0, 0.0)
                pq_lo = pqp.tile([64, QC], F32, tag="pq")
                nc.tensor.matmul(pq_lo, wq_s[:, 0:64], qn01, start=True, stop=True)
                nc.scalar.activation(qf_lo, pq_lo, AF.Identity, bias=bq_s[:, 0:1])
                pq_hi = pqp.tile([64, QC], F32, tag="pq")
                nc.tensor.matmul(pq_hi, wq_s[:, 64:128], qn01, start=True, stop=True)
                nc.scalar.activation(qf_hi, pq_hi, AF.Identity, bias=bq_s[:, 1:2])

            # ---- attention + projection + MLP ----
            with tc.tile_pool(name="psum_main", bufs=2, space="PSUM") as pmain, \
                 tc.tile_pool(name="psum_pl", bufs=4, space="PSUM") as pplp, \
                 tc.tile_pool(name="attw", bufs=5) as attw:
                pz = pmain.tile([128, QC], F32, tag="prh")
                for h in range(HEADS):
                    kf = (kf_lo, kf_hi)[h // 2]
                    qf = (qf_lo, qf_hi)[h // 2]
                    hb = 32 * (h % 2)
                    po = pmain.tile([DH + 1, QC], F32, tag="po")
                    for t in range(NKT):
                        pl = pplp.tile([128, QC], F32, tag="pl")
                        nc.tensor.matmul(pl, kf[hb:hb + 32, t, :],
                                         qf[hb:hb + 32, :],
                                         start=True, stop=True)
                        em = attw.tile([128, QC], F32, tag="em")
                        nc.vector.tensor_mul(out=em, in0=pl, in1=Wsb[:, t, :])
                        ee = attw.tile([128, QC], BF16, tag="ee")
                        nc.scalar.activation(ee, em, AF.Exp)
                        ec = attw.tile([128, QC], BF16, tag="ec")
                        eng = nc.gpsimd if h < 2 else nc.vector
                        eng.tensor_mul(out=ec, in0=ee, in1=Csb[:, t, :])
                        nc.tensor.matmul(po, vf[:, t, h, :], ec,
                                         start=(t == 0), stop=(t == NKT - 1))
                    # per-head normalize + projection accumulate
                    rt = work.tile([DH + 1, QC], F32, tag="rt")
                    nc.vector.reciprocal(rt[DH:DH + 1, :], po[DH:DH + 1, :])
                    prh = pmain.tile([DH, QC], F32, tag="prh")
                    nc.tensor.matmul(prh, ones6432[32:33, :], rt[DH:DH + 1, :],
                                     start=True, stop=True)
                    rbh = work.tile([DH, QC], F32, tag="rbh")
                    nc.any.tensor_copy(out=rbh, in_=prh)
                    onh = work.tile([DH, QC], BF16, tag="onh")
                    nc.vector.tensor_mul(out=onh, in0=po[:DH, :], in1=rbh)
                    nc.tensor.matmul(pz, wproj_s[:, h, :], onh,
                                     start=(h == 0), stop=(h == HEADS - 1))

                z0 = io.tile([D, QC], F32, tag="z0")
                nc.scalar.activation(z0, pz, AF.Identity, bias=bproj_s)
                sk = io.tile([D, QC], F32, tag="sk")
                nc.sync.dma_start(sk, skipT[...])
                z = io.tile([D, QC], F32, tag="z")
                nc.vector.tensor_add(out=z, in0=z0, in1=sk)

                zf = io.tile([D, QC], F32, tag="zf")
                _ln_partition_stats(nc, work, io, pmain, "prh", pmain, "po",
                                    ones_col, ones_row, z, zf, preg_s, preb_s)
                zfb = io.tile([D, QC], BF16, tag="zfb")
                nc.any.tensor_copy(out=zfb, in_=zf)

                h1 = io.tile([D, 2, QC], BF16, tag="h1")
                for j in range(2):
                    ph = pplp.tile([128, QC], F32, tag="pl")
                    nc.tensor.matmul(ph, w1_s[:, 128 * j:128 * (j + 1)], zfb,
                                     start=True, stop=True)
                    nc.scalar.activation(h1[:, j, :], ph, AF.Gelu,
                                         bias=b1_s[:, j:j + 1])
                pm = pplp.tile([128, QC], F32, tag="pl")
                nc.tensor.matmul(pm, w2_s[:, 0, :], h1[:, 0, :], start=True,
                                 stop=False)
                nc.tensor.matmul(pm, w2_s[:, 1, :], h1[:, 1, :], start=False,
                                 stop=True)
                z2 = io.tile([D, QC], F32, tag="z2")
                nc.scalar.activation(z2, pm, AF.Identity, bias=b2_s)
                z3 = io.tile([D, QC], F32, tag="z3")
                nc.vector.tensor_add(out=z3, in0=z2, in1=zf)

                zo = io.tile([D, QC], F32, tag="zo")
                _ln_partition_stats(nc, work, io, pmain, "prh", pmain, "po",
                                    ones_col, ones_row, z3, zo, postg_s, postb_s)
                nc.sync.dma_start(outT[...], zo)

    nc.finalize()
    _CACHE["nc"] = nc
    return nc


def _prep_inputs(inputs):
    f32 = np.float32
    bf16 = ml_dtypes.bfloat16
    q = np.asarray(inputs["q"], f32)
    k = np.asarray(inputs["k"], f32)
    v = np.asarray(inputs["v"], f32)
    W = np.asarray(inputs["W_logits"], f32)
    vis = np.asarray(inputs["vis"])
    skip = np.asarray(inputs["skip"], f32)

    g = lambda n: np.asarray(inputs[n], f32)
    qn_g, qn_b = g("qn_g"), g("qn_b")
    kn_g, kn_b = g("kn_g"), g("kn_b")
    vn_g, vn_b = g("vn_g"), g("vn_b")
    wq, bq = g("wq"), g("bq")
    wk, bk = g("wk"), g("bk")
    wv, bv = g("wv"), g("bv")
    wproj, bproj = g("wproj"), g("bproj")
    pre_g, pre_b = g("pre_g"), g("pre_b")
    w1, b1 = g("w1"), g("b1")
    w2, b2 = g("w2"), g("b2")
    post_g, post_b = g("post_g"), g("post_b")

    # fold LN affine params into projections; fold attention scale into q
    wq2 = (wq * qn_g[None, :]) * SCALE
    bq2 = (wq @ qn_b + bq) * SCALE
    wk2 = wk * kn_g[None, :]
    bk2 = wk @ kn_b + bk
    wv2 = wv * vn_g[None, :]
    bv2 = wv @ vn_b + bv

    # q/skip -> [D, Q] padded
    qT = np.zeros((D, QPAD), f32)
    qT[:, :Q] = q.reshape(D, Q)
    skipT = np.zeros((D, QPAD), f32)
    skipT[:, :Q] = skip.reshape(D, Q)

    # k/v -> rows [NKP, D]
    kRow = np.zeros((NKP, D), f32)
    kRow[:NK] = np.transpose(k, (0, 1, 3, 4, 2)).reshape(NK, D)
    vRow = np.zeros((NKP, D), f32)
    vRow[:NK] = np.transpose(v, (0, 1, 3, 4, 2)).reshape(NK, D)

    # W/vis -> transposed, padded; vis pad rows (queries) with 1 to avoid
    # a zero softmax denominator in the padding region
    Wp = np.zeros((QPAD, NKP), f32)
    Wp[:Q, :NK] = W[0]
    Cp = np.zeros((QPAD, NKP), f32)
    Cp[:Q, :NK] = vis[0]
    Cp[Q:, :] = 1.0

    # wproj head-major: wprojT [inner, D] -> [DH, HEADS, D]
    wprojT = np.ascontiguousarray(wproj.T)         # [inner, D]
    wprojTm = np.ascontiguousarray(
        wprojT.reshape(HEADS, DH, D).transpose(1, 0, 2))  # [DH, HEADS, D]

    shared = {
        "kR": kRow,
        "vR": vRow,
        "wqT": np.ascontiguousarray(wq2.T).astype(bf16),
        "wkT": np.ascontiguousarray(wk2.T).astype(bf16),
        "wvT": np.ascontiguousarray(wv2.T).astype(bf16),
        "bqm": np.ascontiguousarray(bq2.reshape(2, 64).T),
        "bkm": np.ascontiguousarray(bk2.reshape(2, 64).T),
        "wprojTm": wprojTm.astype(bf16),
        "bprojv": np.ascontiguousarray((wproj @ bv2 + bproj)[:, None]),
        "pre_gv": np.ascontiguousarray(pre_g[:, None]),
        "pre_bv": np.ascontiguousarray(pre_b[:, None]),
        "w1T": np.ascontiguousarray(w1.T).astype(bf16),
        "b1m": np.ascontiguousarray(b1.reshape(2, D).T),
        "w2Td": np.ascontiguousarray(w2.T.reshape(2, D, D)).astype(bf16),
        "b2v": np.ascontiguousarray(b2[:, None]),
        "post_gv": np.ascontiguousarray(post_g[:, None]),
        "post_bv": np.ascontiguousarray(post_b[:, None]),
    }

    in_maps = []
    for c in range(NCORES):
        sl = slice(c * QC, (c + 1) * QC)
        m = dict(shared)
        m["qT"] = np.ascontiguousarray(qT[:, sl])
        m["skipT"] = np.ascontiguousarray(skipT[:, sl])
        m["Wt"] = np.ascontiguousarray(Wp[sl].T).reshape(NKT, 128, QC).astype(bf16)
        m["Cm"] = np.ascontiguousarray(Cp[sl].T).reshape(NKT, 128, QC).astype(bf16)
        in_maps.append(m)
    return in_maps


def kernel(**inputs):
    from concourse.bass_utils import run_bass_kernel_spmd

    nc = _build()
    in_maps = _prep_inputs(inputs)
    res = run_bass_kernel_spmd(nc, in_maps, core_ids=list(range(NCORES)))
    outs = np.concatenate([r["outT"] for r in res.results], axis=1)  # [D, QPAD]
    return outs[:, :Q].reshape(1, D, HB, WB).astype(np.float32)



# revision 14
# speedup vs baseline: 1.3045x; 1.3045x over previous
"""CrossViewAttention Trainium2 kernel, v2.

Sharding: Q=2500 queries split across 8 cores (QC=320 each, padded).
Softmax is over NK (local per core) so no collectives.

Key structure (per core), all in the "transposed" attention layout
(nk on partitions, q on free):

- k is consumed RAW in column-major [D, NK]: the logits matmul is
  pl = kT_tile^T @ qk_h where qk_h = WkC_h^T @ qf absorbs the k-side
  projection.  WkC is row-centered on the host, which implements the
  k-LayerNorm mean subtraction exactly; the k-LN 1/std rides the ACT
  engine's per-partition `scale` operand of the exp:
      ee = Exp(rstd_k * (pl ⊙ (W*vis)))
- v is consumed RAW in row-major [NK, D]; its LayerNorm is applied as
  vS = (vR - mean)·rstd on the (otherwise idle) GPSIMD engine.  The
  attention output stays in raw d-space (num' = vS^T @ ee) and the
  fused (wproj_h @ wv_h) matrix maps it to the output at the end.
- The `· vis` mask multiply after the exp is removed entirely via
      exp(x)·vis = ee - 1 + vis      (exact: x=0 where vis=0)
  The "-1+vis" part becomes one extra PE matmul per tile against the
  host tensor (vis-1), and a host-computed denominator correction.
- Softmax denominators accumulate via M=1 ones-matmuls col-packed into
  one PSUM bank; heads run in two passes of two so PSUM fits exactly.
- One ACT table set (natural_log_exp_and_others) serves the whole
  kernel: exp for attention, ln+exp for every 1/sqrt in the LNs, and
  gelu is evaluated as an odd polynomial on the DVE.
"""

import math
import sys

if "/opt/trn_rl_repo" not in sys.path:
    sys.path.insert(0, "/opt/trn_rl_repo")

import numpy as np
import ml_dtypes

import concourse.bass as bass
import concourse.bacc as bacc_mod
import concourse.mybir as mybir
from concourse.tile import TileContext

HEADS = 4
DH = 32
D = 128
EPS = 1e-5
HB = WB = 50
Q = HB * WB            # 2500
NVIEW, KH, KW = 6, 24, 44
NK = NVIEW * KH * KW   # 6336
NCORES = 8
QC = 320               # queries per core (Q padded to 2560)
QPAD = NCORES * QC
NKP = 6400             # NK padded to 50*128
NKT = NKP // 128       # 50 nk tiles
SCALE = DH ** -0.5

F32 = mybir.dt.float32
BF16 = mybir.dt.bfloat16
AF = mybir.ActivationFunctionType
ALU = mybir.AluOpType

_CACHE = {}

# odd-polynomial fit of Phi(x)-0.5 = x*(GA + GB*x^2 + GC*x^4) on [-2.8, 2.8]
def _fit_gelu():
    xs = np.linspace(-2.8, 2.8, 4001)
    xs = xs[np.abs(xs) > 1e-3]
    import math as _m
    phi = np.array([0.5 * (1.0 + _m.erf(t / _m.sqrt(2.0))) for t in xs])
    z = (phi - 0.5) / xs
    y = xs * xs
    Amat = np.stack([np.ones_like(y), y, y * y], axis=1)
    coef, *_ = np.linalg.lstsq(Amat, z, rcond=None)
    return [float(c) for c in coef]

GA, GB, GC = _fit_gelu()


def _ln_partition(nc, work, ps_pool, ones_col, ones_row, eps_c, x, out,
                  g_ap, b_ap):
    """LayerNorm over the PARTITION dim of x [128, Qf] -> out (any dtype).

    Column stats via ones-matmuls; 1/sqrt via exp(-0.5*ln(var+eps));
    broadcast back via K=1 matmuls; optional per-partition affine g/b.
    """
    Qf = x.shape[-1]
    sq = work.tile([128, Qf], F32, tag="ln_sq")
    nc.scalar.activation(sq, x, AF.Square)
    # one psum bank for both stats (sequential reuse via same tag)
    s1 = ps_pool.tile([1, Qf], F32, tag="ln_s")
    nc.tensor.matmul(s1, ones_col, x, start=True, stop=True)
    mean = work.tile([1, Qf], F32, tag="ln_mean")
    nc.scalar.mul(mean, s1, 1.0 / 128.0)
    s2 = ps_pool.tile([1, Qf], F32, tag="ln_s")
    nc.tensor.matmul(s2, ones_col, sq, start=True, stop=True)
    ex2 = work.tile([1, Qf], F32, tag="ln_ex2")
    nc.scalar.mul(ex2, s2, 1.0 / 128.0)
    m2 = work.tile([1, Qf], F32, tag="ln_m2")
    nc.vector.tensor_mul(out=m2, in0=mean, in1=mean)
    var = work.tile([1, Qf], F32, tag="ln_var")
    nc.vector.tensor_tensor(out=var, in0=ex2, in1=m2, op=ALU.subtract)
    lnv = work.tile([1, Qf], F32, tag="ln_lnv")
    nc.scalar.activation(lnv, var, AF.Ln, bias=eps_c[0:1])
    rstd = work.tile([1, Qf], F32, tag="ln_rstd")
    nc.scalar.activation(rstd, lnv, AF.Exp, scale=-0.5)
    nmr = work.tile([1, Qf], F32, tag="ln_nmr")
    nc.vector.tensor_mul(out=nmr, in0=mean, in1=rstd)
    nc.scalar.mul(nmr, nmr, -1.0)
    # one psum bank for both broadcasts (sequential reuse via same tag)
    rstdB = ps_pool.tile([128, Qf], F32, tag="ln_b")
    nc.tensor.matmul(rstdB, ones_row, rstd, start=True, stop=True)
    t1 = work.tile([128, Qf], F32, tag="ln_t1")
    nc.vector.tensor_mul(out=t1, in0=x, in1=rstdB)
    nmrB = ps_pool.tile([128, Qf], F32, tag="ln_b")
    nc.tensor.matmul(nmrB, ones_row, nmr, start=True, stop=True)
    if g_ap is None:
        nc.vector.tensor_add(out=out, in0=t1, in1=nmrB)
    else:
        t2 = work.tile([128, Qf], F32, tag="ln_t2")
        nc.vector.tensor_add(out=t2, in0=t1, in1=nmrB)
        nc.scalar.activation(out, t2, AF.Identity, scale=g_ap, bias=b_ap)


def _build():
    if "nc" in _CACHE:
        return _CACHE["nc"]
    nc = bacc_mod.Bacc()

    # ---- I/O ----
    qT = nc.dram_tensor("qT", [D, QC], F32, kind="ExternalInput")
    skipT = nc.dram_tensor("skipT", [D, QC], F32, kind="ExternalInput")
    kTd = nc.dram_tensor("kTd", [D, NKP], BF16, kind="ExternalInput")
    vRd = nc.dram_tensor("vRd", [NKP, D], BF16, kind="ExternalInput")
    Wvd = nc.dram_tensor("Wvd", [NKT, 128, QC], BF16, kind="ExternalInput")
    Vm1d = nc.dram_tensor("Vm1d", [NKT, 128, QC], BF16, kind="ExternalInput")
    dcord = nc.dram_tensor("dcord", [D, QC], F32, kind="ExternalInput")
    Wq2Td = nc.dram_tensor("Wq2Td", [D, D], BF16, kind="ExternalInput")
    bq2d = nc.dram_tensor("bq2d", [D, 1], F32, kind="ExternalInput")
    WkC4d = nc.dram_tensor("WkC4d", [D, D], BF16, kind="ExternalInput")
    WCTd = nc.dram_tensor("WCTd", [D, HEADS, D], BF16, kind="ExternalInput")
    bprojd = nc.dram_tensor("bprojd", [D, 1], F32, kind="ExternalInput")
    pre_gd = nc.dram_tensor("pre_gd", [D, 1], F32, kind="ExternalInput")
    pre_bd = nc.dram_tensor("pre_bd", [D, 1], F32, kind="ExternalInput")
    w1Td = nc.dram_tensor("w1Td", [D, 2 * D], BF16, kind="ExternalInput")
    b1d = nc.dram_tensor("b1d", [D, 2], F32, kind="ExternalInput")
    w2Tdd = nc.dram_tensor("w2Tdd", [2, D, D], BF16, kind="ExternalInput")
    b2d = nc.dram_tensor("b2d", [D, 1], F32, kind="ExternalInput")
    post_gd = nc.dram_tensor("post_gd", [D, 1], F32, kind="ExternalInput")
    post_bd = nc.dram_tensor("post_bd", [D, 1], F32, kind="ExternalInput")
    outT = nc.dram_tensor("outT", [D, QC], F32, kind="ExternalOutput")

    with TileContext(nc) as tc:
        with tc.tile_pool(name="const", bufs=1) as cpool, \
             tc.tile_pool(name="big", bufs=1) as bigpool, \
             tc.tile_pool(name="work", bufs=3) as work, \
             tc.tile_pool(name="io", bufs=1) as io:

            # ---- constants ----
            ones_col = cpool.tile([128, 1], F32)
            nc.any.memset(ones_col, 1.0)
            ones_row = cpool.tile([1, 128], F32)
            nc.any.memset(ones_row, 1.0)
            ones_rows_f = cpool.tile([128, 128], F32)
            nc.any.memset(ones_rows_f, 1.0)
            onesb_col = cpool.tile([128, 1], BF16)
            nc.any.memset(onesb_col, 1.0)
            invd_col = cpool.tile([128, 1], BF16)
            nc.any.memset(invd_col, 1.0 / 128.0)
            zero_c = cpool.tile([128, 1], F32)
            nc.any.memset(zero_c, 0.0)
            nc.const_aps.aps[(F32, 0.0)] = zero_c[:]
            eps_c = cpool.tile([128, 1], F32)
            nc.any.memset(eps_c, EPS)
            nc.const_aps.aps[(F32, EPS)] = eps_c[:]

            def load_const(dram, shape, dt):
                t = cpool.tile(shape, dt, tag="c_" + dram.name)
                nc.sync.dma_start(t, dram[...])
                return t

            Wq2T_s = load_const(Wq2Td, [D, D], BF16)
            bq2_s = load_const(bq2d, [D, 1], F32)
            WkC4_s = load_const(WkC4d, [D, D], BF16)
            WCT_s = load_const(WCTd, [D, HEADS, D], BF16)
            bproj_s = load_const(bprojd, [D, 1], F32)
            preg_s = load_const(pre_gd, [D, 1], F32)
            preb_s = load_const(pre_bd, [D, 1], F32)
            w1_s = load_const(w1Td, [D, 2 * D], BF16)
            b1_s = load_const(b1d, [D, 2], F32)
            w2_s = cpool.tile([D, 2, D], BF16)
            nc.sync.dma_start(w2_s[:, 0, :], w2Tdd[0])
            nc.sync.dma_start(w2_s[:, 1, :], w2Tdd[1])
            b2_s = load_const(b2d, [D, 1], F32)
            postg_s = load_const(post_gd, [D, 1], F32)
            postb_s = load_const(post_bd, [D, 1], F32)
            dcor_s = load_const(dcord, [D, QC], F32)

            # ---- big resident tensors ----
            kT_s = bigpool.tile([128, NKP], BF16)      # raw k, column-major
            vR_s = bigpool.tile([128, NKT, D], BF16)   # raw v, row-major
            vS_s = bigpool.tile([128, NKT, D], BF16)   # LayerNormed v
            Wv_s = bigpool.tile([128, NKT, QC], BF16)  # W*vis, transposed
            Vm1_s = bigpool.tile([128, NKT, QC], BF16)  # vis-1
            qk4_sb = bigpool.tile([128, HEADS, QC], BF16)
            rstdk = bigpool.tile([128, NKT], F32)
            A_sb = bigpool.tile([128, HEADS, QC], BF16)
            B_sb = bigpool.tile([128, QC], BF16)

            # DMA of the big streams (chunked)
            KCH = 1600
            for c in range(0, NKP, KCH):
                nc.sync.dma_start(kT_s[:, c:c + KCH], kTd[:, c:c + KCH])
            VCH = 10
            for c0 in range(0, NKT, VCH):
                nc.sync.dma_start(
                    vR_s[:, c0:c0 + VCH, :],
                    vRd[c0 * 128:(c0 + VCH) * 128, :].rearrange(
                        "(t p) d -> p t d", p=128))
            for t in range(NKT):
                nc.sync.dma_start(Wv_s[:, t, :], Wvd[t])
            for t in range(NKT):
                nc.sync.dma_start(Vm1_s[:, t, :], Vm1d[t])

            # ---- prep: k stats, v stats+normalize, q path ----
            with tc.tile_pool(name="ps_prep", bufs=1, space="PSUM") as psp, \
                 tc.tile_pool(name="ps_prep2", bufs=1, space="PSUM") as psp2, \
                 tc.tile_pool(name="prepw", bufs=2) as prepw:
                # k^2 on gpsimd (kT is bf16; square chunked)
                kTsq = bigpool.tile([128, NKP], BF16, tag="kTsq")
                for c in range(0, NKP, KCH):
                    nc.gpsimd.tensor_tensor(
                        out=kTsq[:, c:c + KCH], in0=kT_s[:, c:c + KCH],
                        in1=kT_s[:, c:c + KCH], op=ALU.mult)
                # per-tile column sums -> [128nk, 1] slots, one bank reused
                s1p = psp.tile([128, NKT], F32, tag="sstat")
                for t in range(NKT):
                    nc.tensor.matmul(s1p[:, t:t + 1],
                                     kT_s[:, t * 128:(t + 1) * 128],
                                     invd_col, start=True, stop=True)
                s1s = prepw.tile([128, NKT], F32, tag="s1s")
                nc.scalar.copy(s1s, s1p)
                s2p = psp.tile([128, NKT], F32, tag="sstat")
                for t in range(NKT):
                    nc.tensor.matmul(s2p[:, t:t + 1],
                                     kTsq[:, t * 128:(t + 1) * 128],
                                     invd_col, start=True, stop=True)
                s2s = prepw.tile([128, NKT], F32, tag="s2s")
                nc.scalar.copy(s2s, s2p)
                mk2 = prepw.tile([128, NKT], F32, tag="mk2")
                nc.vector.tensor_mul(out=mk2, in0=s1s, in1=s1s)
                vark = prepw.tile([128, NKT], F32, tag="vark")
                nc.vector.tensor_tensor(out=vark, in0=s2s, in1=mk2,
                                        op=ALU.subtract)
                lnvk = prepw.tile([128, NKT], F32, tag="lnvk")
                nc.scalar.activation(lnvk, vark, AF.Ln, bias=eps_c)
                nc.scalar.activation(rstdk, lnvk, AF.Exp, scale=-0.5)

                # v stats via bn_stats (one group of 128 per call)
                bsv = prepw.tile([128, NKT, 6], F32, tag="bsv")
                for t in range(NKT):
                    nc.vector.bn_stats(bsv[:, t, :], vR_s[:, t, :])
                me = bsv[:, :, 1]
                mo = bsv[:, :, 4]
                m2e = bsv[:, :, 2]
                m2o = bsv[:, :, 5]
                sv = prepw.tile([128, NKT], F32, tag="sv")
                nc.vector.tensor_add(out=sv, in0=me, in1=mo)
                meanv = prepw.tile([128, NKT], F32, tag="meanv")
                nc.scalar.mul(meanv, sv, 0.5)
                tee = prepw.tile([128, NKT], F32, tag="tee")
                nc.vector.tensor_mul(out=tee, in0=me, in1=me)
                too = prepw.tile([128, NKT], F32, tag="too")
                nc.vector.tensor_mul(out=too, in0=mo, in1=mo)
                sum2 = prepw.tile([128, NKT], F32, tag="sum2")
                nc.vector.tensor_add(out=sum2, in0=tee, in1=too)
                m2s = prepw.tile([128, NKT], F32, tag="m2s")
                nc.vector.tensor_add(out=m2s, in0=m2e, in1=m2o)
                ex2v = prepw.tile([128, NKT], F32, tag="ex2v")
                nc.vector.tensor_scalar(out=ex2v, in0=m2s,
                                        scalar1=1.0 / 128.0, scalar2=None,
                                        op0=ALU.mult)
                sum2h = prepw.tile([128, NKT], F32, tag="sum2h")
                nc.scalar.mul(sum2h, sum2, 0.5)
                nc.vector.tensor_add(out=ex2v, in0=ex2v, in1=sum2h)
                mv2 = prepw.tile([128, NKT], F32, tag="mv2")
                nc.vector.tensor_mul(out=mv2, in0=meanv, in1=meanv)
                varv = prepw.tile([128, NKT], F32, tag="varv")
                nc.vector.tensor_tensor(out=varv, in0=ex2v, in1=mv2,
                                        op=ALU.subtract)
                lnvv = prepw.tile([128, NKT], F32, tag="lnvv")
                nc.scalar.activation(lnvv, varv, AF.Ln, bias=eps_c)
                rstdv = prepw.tile([128, NKT], F32, tag="rstdv")
                nc.scalar.activation(rstdv, lnvv, AF.Exp, scale=-0.5)
                mtil = prepw.tile([128, NKT], F32, tag="mtil")
                nc.vector.tensor_mul(out=mtil, in0=meanv, in1=rstdv)
                # vS = vR*rstdv - mtil on gpsimd, chunked
                for c0 in range(0, NKT, VCH):
                    tmpv = prepw.tile([128, VCH, D], BF16, tag="tmpv")
                    nc.gpsimd.tensor_tensor(
                        out=tmpv, in0=vR_s[:, c0:c0 + VCH, :],
                        in1=rstdv[:, c0:c0 + VCH, None].to_broadcast(
                            (128, VCH, D)), op=ALU.mult)
                    nc.gpsimd.tensor_tensor(
                        out=vS_s[:, c0:c0 + VCH, :], in0=tmpv,
                        in1=mtil[:, c0:c0 + VCH, None].to_broadcast(
                            (128, VCH, D)), op=ALU.subtract)

                # q path: LN + projection + per-head k-mix
                qsb = io.tile([D, QC], F32, tag="qsb")
                nc.sync.dma_start(qsb, qT[...])
                qn = io.tile([D, QC], BF16, tag="qn")
                _ln_partition(nc, work, psp2, ones_col, ones_row, eps_c,
                              qsb, qn, None, None)
                qfp = psp2.tile([128, QC], F32, tag="ln_b")
                nc.tensor.matmul(qfp, Wq2T_s, qn, start=True, stop=True)
                qf_sb = io.tile([D, QC], BF16, tag="qf_sb")
                nc.scalar.activation(qf_sb, qfp, AF.Identity, bias=bq2_s)
                qk4p = psp.tile([128, HEADS, 512], F32, tag="qk4p")
                for h in range(HEADS):
                    nc.tensor.matmul(qk4p[:, h, 0:QC],
                                     WkC4_s[32 * h:32 * h + 32, :],
                                     qf_sb[32 * h:32 * h + 32, :],
                                     start=True, stop=True,
                                     tile_position=(32 * h, 0))
                nc.scalar.copy(qk4_sb, qk4p[:, :, 0:QC])

            # ---- attention: two passes of two heads ----
            with tc.tile_pool(name="ps_pl", bufs=2, space="PSUM") as ps_pl, \
                 tc.tile_pool(name="ps_acc", bufs=1, space="PSUM") as ps_acc, \
                 tc.tile_pool(name="ps_bd", bufs=1, space="PSUM") as ps_bd, \
                 tc.tile_pool(name="attw", bufs=3) as attw:
                den_t = ps_bd.tile([128, QC], F32, tag="den")
                ones_qc = work.tile([128, QC], F32, tag="ones_qc")
                nc.any.memset(ones_qc, 1.0)
                nc.scalar.copy(den_t, ones_qc)
                B_t = ps_bd.tile([128, QC], F32, tag="B")
                for hp in range(2):
                    A0 = ps_acc.tile([128, QC], F32, tag="A0")
                    A1 = ps_acc.tile([128, QC], F32, tag="A1")
                    for t in range(NKT):
                        ksl = kT_s[:, t * 128:(t + 1) * 128]
                        pl = ps_pl.tile([128, 2, 512], F32, tag="pl")
                        for i in range(2):
                            nc.tensor.matmul(pl[:, i, 0:QC], ksl,
                                             qk4_sb[:, 2 * hp + i, :],
                                             start=True, stop=True)
                        em = attw.tile([128, 2, QC], BF16, tag="em")
                        nc.vector.tensor_mul(
                            out=em, in0=pl[:, :, 0:QC],
                            in1=Wv_s[:, t, None, :].to_broadcast(
                                (128, 2, QC)))
                        ee = attw.tile([128, 2, QC], BF16, tag="ee")
                        nc.scalar.activation(ee, em, AF.Exp,
                                             scale=rstdk[:, t:t + 1])
                        vsl = vS_s[:, t, :]
                        nc.tensor.matmul(A0, vsl, ee[:, 0, :],
                                         start=(t == 0), stop=(t == NKT - 1))
                        nc.tensor.matmul(A1, vsl, ee[:, 1, :],
                                         start=(t == 0), stop=(t == NKT - 1))
                        if hp == 0:
                            nc.tensor.matmul(B_t, vsl, Vm1_s[:, t, :],
                                             start=(t == 0),
                                             stop=(t == NKT - 1))
                        for i in range(2):
                            h = 2 * hp + i
                            nc.tensor.matmul(den_t[32 * h:32 * h + 1, :],
                                             onesb_col, ee[:, i, :],
                                             start=(t == 0),
                                             stop=(t == NKT - 1),
                                             tile_position=(0, 32 * h),
                                             skip_group_check=True)
                    nc.scalar.copy(A_sb[:, 2 * hp, :], A0)
                    nc.scalar.copy(A_sb[:, 2 * hp + 1, :], A1)
                    if hp == 0:
                        nc.scalar.copy(B_sb, B_t)

                # ---- denominators ----
                den2 = io.tile([128, QC], F32, tag="den2")
                nc.vector.tensor_add(out=den2, in0=den_t, in1=dcor_s)
                rden = io.tile([128, QC], F32, tag="rden")
                nc.vector.reciprocal(rden, den2)

            # ---- projection + residual + MLP tail ----
            with tc.tile_pool(name="ps_tail", bufs=1, space="PSUM") as pst, \
                 tc.tile_pool(name="ps_tail2", bufs=1, space="PSUM") as pst2, \
                 tc.tile_pool(name="tailw", bufs=2) as tailw:
                pz = pst.tile([128, QC], F32, tag="pz")
                for h in range(HEADS):
                    rdb = pst2.tile([128, QC], F32, tag="rdb")
                    nc.tensor.matmul(rdb,
                                     ones_rows_f[32 * h:32 * h + 1, :],
                                     rden[32 * h:32 * h + 1, :],
                                     start=True, stop=True,
                                     tile_position=(32 * h, 0))
                    ab = tailw.tile([128, QC], BF16, tag="ab")
                    nc.gpsimd.tensor_add(out=ab, in0=A_sb[:, h, :], in1=B_sb)
                    onh = tailw.tile([128, QC], BF16, tag="onh")
                    nc.vector.tensor_mul(out=onh, in0=ab, in1=rdb)
                    nc.tensor.matmul(pz, WCT_s[:, h, :], onh,
                                     start=(h == 0), stop=(h == HEADS - 1))
                z0 = io.tile([D, QC], F32, tag="z0")
                nc.scalar.activation(z0, pz, AF.Identity, bias=bproj_s)
                sk = io.tile([D, QC], F32, tag="sk")
                nc.sync.dma_start(sk, skipT[...])
                z = io.tile([D, QC], F32, tag="z")
                nc.vector.tensor_add(out=z, in0=z0, in1=sk)

                zf = io.tile([D, QC], F32, tag="zf")
                _ln_partition(nc, work, pst2, ones_col, ones_row, eps_c,
                              z, zf, preg_s, preb_s)
                zfb = io.tile([D, QC], BF16, tag="zfb")
                nc.any.tensor_copy(out=zfb, in_=zf)

                # MLP: h1 = W1 @ zf + b1 ; gelu via DVE polynomial
                x1 = io.tile([D, 2, QC], BF16, tag="x1")
                for j in range(2):
                    ph = pst2.tile([128, QC], F32, tag="ph")
                    nc.tensor.matmul(ph, w1_s[:, 128 * j:128 * (j + 1)], zfb,
                                     start=True, stop=True)
                    nc.scalar.activation(x1[:, j, :], ph, AF.Identity,
                                         bias=b1_s[:, j:j + 1])
                # gelu(x) = x*(0.5 + x*(GA + GB*y + GC*y^2)), y = x^2
                y1 = io.tile([D, 2, QC], BF16, tag="y1")
                nc.vector.tensor_mul(out=y1, in0=x1, in1=x1)
                p1 = io.tile([D, 2, QC], BF16, tag="p1")
                nc.vector.tensor_scalar(out=p1, in0=y1, scalar1=GC,
                                        scalar2=GB, op0=ALU.mult, op1=ALU.add)
                p2 = io.tile([D, 2, QC], BF16, tag="p2")
                nc.vector.tensor_mul(out=p2, in0=p1, in1=y1)
                p3 = io.tile([D, 2, QC], BF16, tag="p3")
                nc.vector.tensor_scalar(out=p3, in0=p2, scalar1=GA,
                                        scalar2=None, op0=ALU.add)
                p4 = io.tile([D, 2, QC], BF16, tag="p4")
                nc.vector.tensor_mul(out=p4, in0=p3, in1=x1)
                p5 = io.tile([D, 2, QC], BF16, tag="p5")
                nc.vector.tensor_scalar(out=p5, in0=p4, scalar1=0.5,
                                        scalar2=None, op0=ALU.add)
                h1 = io.tile([D, 2, QC], BF16, tag="h1")
                nc.vector.tensor_mul(out=h1, in0=p5, in1=x1)

                pm = pst.tile([128, QC], F32, tag="pm")
                nc.tensor.matmul(pm, w2_s[:, 0, :], h1[:, 0, :], start=True,
                                 stop=False)
                nc.tensor.matmul(pm, w2_s[:, 1, :], h1[:, 1, :], start=False,
                                 stop=True)
                z2 = io.tile([D, QC], F32, tag="z2")
                nc.scalar.activation(z2, pm, AF.Identity, bias=b2_s)
                z3 = io.tile([D, QC], F32, tag="z3")
                nc.vector.tensor_add(out=z3, in0=z2, in1=zf)

                zo = io.tile([D, QC], F32, tag="zo")
                _ln_partition(nc, work, pst2, ones_col, ones_row, eps_c,
                              z3, zo, postg_s, postb_s)
                nc.sync.dma_start(outT[...], zo)

    nc.finalize()
    _CACHE["nc"] = nc
    return nc


def _prep_inputs(inputs):
    f32 = np.float32
    bf16 = ml_dtypes.bfloat16
    q = np.asarray(inputs["q"], f32)
    k = np.asarray(inputs["k"], f32)
    v = np.asarray(inputs["v"], f32)
    W = np.asarray(inputs["W_logits"], f32)
    vis = np.asarray(inputs["vis"])
    skip = np.asarray(inputs["skip"], f32)

    g = lambda n: np.asarray(inputs[n], f32)
    qn_g, qn_b = g("qn_g"), g("qn_b")
    kn_g, kn_b = g("kn_g"), g("kn_b")
    vn_g, vn_b = g("vn_g"), g("vn_b")
    wq, bq = g("wq"), g("bq")
    wk, bk = g("wk"), g("bk")
    wv, bv = g("wv"), g("bv")
    wproj, bproj = g("wproj"), g("bproj")
    pre_g, pre_b = g("pre_g"), g("pre_b")
    w1, b1 = g("w1"), g("b1")
    w2, b2 = g("w2"), g("b2")
    post_g, post_b = g("post_g"), g("post_b")

    # fold LN affine params into projections; attention scale into q side
    wq2 = (wq * qn_g[None, :]) * SCALE
    bq2 = (wq @ qn_b + bq) * SCALE
    wk2 = wk * kn_g[None, :]
    bk2 = wk @ kn_b + bk
    assert np.abs(bk2).max() < 1e-6, "nonzero k-bias not supported"
    wv2 = wv * vn_g[None, :]
    bv2 = wv @ vn_b + bv

    # WkC: row-centered k projection (implements k-LN mean subtraction)
    WkC = wk2 - wk2.mean(axis=1, keepdims=True)          # [inner, D]
    # WC_h = wproj[:, h] @ wv2[h]  -> lhsT layout [d, h, o]
    WCT = np.zeros((D, HEADS, D), f32)
    for h in range(HEADS):
        WC_h = wproj[:, 32 * h:32 * h + 32] @ wv2[32 * h:32 * h + 32, :]
        WCT[:, h, :] = WC_h.T
    bprojv = (wproj @ bv2 + bproj)[:, None]

    # q/skip -> [D, Q] padded
    qTf = np.zeros((D, QPAD), f32)
    qTf[:, :Q] = q.reshape(D, Q)
    skipTf = np.zeros((D, QPAD), f32)
    skipTf[:, :Q] = skip.reshape(D, Q)

    # k -> [D, NKP]; v -> [NKP, D]
    kT = np.zeros((D, NKP), f32)
    kT[:, :NK] = np.transpose(k, (0, 2, 1, 3, 4)).reshape(D, NK)
    vRow = np.zeros((NKP, D), f32)
    vRow[:NK] = np.transpose(v, (0, 1, 3, 4, 2)).reshape(NK, D)

    # masks, transposed+padded
    visf = vis[0].astype(f32)                            # [Q, NK]
    Wp = np.zeros((QPAD, NKP), f32)
    Wp[:Q, :NK] = W[0] * visf
    Vm1 = np.zeros((QPAD, NKP), f32)
    Vm1[:Q, :NK] = visf - 1.0
    Vm1[:Q, NK:] = -1.0        # padded keys are masked for real queries
    # padded queries: fully visible (Vm1=0, Wv=0) -> den=NKP, no div0
    dencorr = Vm1.sum(axis=1)                            # [QPAD]
    dcor = np.zeros((D, QPAD), f32)
    for h in range(HEADS):
        dcor[32 * h, :] = dencorr

    shared = {
        "kTd": kT.astype(bf16),
        "vRd": vRow.astype(bf16),
        "Wq2Td": np.ascontiguousarray(wq2.T).astype(bf16),
        "bq2d": np.ascontiguousarray(bq2[:, None]),
        "WkC4d": WkC.astype(bf16),
        "WCTd": WCT.astype(bf16),
        "bprojd": np.ascontiguousarray(bprojv),
        "pre_gd": np.ascontiguousarray(pre_g[:, None]),
        "pre_bd": np.ascontiguousarray(pre_b[:, None]),
        "w1Td": np.ascontiguousarray(w1.T).astype(bf16),
        "b1d": np.ascontiguousarray(b1.reshape(2, D).T),
        "w2Tdd": np.ascontiguousarray(w2.T.reshape(2, D, D)).astype(bf16),
        "b2d": np.ascontiguousarray(b2[:, None]),
        "post_gd": np.ascontiguousarray(post_g[:, None]),
        "post_bd": np.ascontiguousarray(post_b[:, None]),
    }

    in_maps = []
    for c in range(NCORES):
        sl = slice(c * QC, (c + 1) * QC)
        m = dict(shared)
        m["qT"] = np.ascontiguousarray(qTf[:, sl])
        m["skipT"] = np.ascontiguousarray(skipTf[:, sl])
        m["Wvd"] = np.ascontiguousarray(
            Wp[sl].T).reshape(NKT, 128, QC).astype(bf16)
        m["Vm1d"] = np.ascontiguousarray(
            Vm1[sl].T).reshape(NKT, 128, QC).astype(bf16)
        m["dcord"] = np.ascontiguousarray(dcor[:, sl])
        in_maps.append(m)
    return in_maps


def kernel(**inputs):
    from concourse.bass_utils import run_bass_kernel_spmd

    nc = _build()
    in_maps = _prep_inputs(inputs)
    res = run_bass_kernel_spmd(nc, in_maps, core_ids=list(range(NCORES)))
    outs = np.concatenate([r["outT"] for r in res.results], axis=1)
    return outs[:, :Q].reshape(1, D, HB, WB).astype(np.float32)
